# revision 36
# baseline (speedup 1.0000x reference)
"""Trainium2 Bass kernel for an encoder-decoder (S2S) transformer.

Distribution: 8 NeuronCores = 4 data-parallel groups (batch B=4) x 2-way
SEQUENCE-parallel within each pair.  Each core owns 256 tokens (2 tiles of
128) of one batch element at full model width, so layernorm, FFN and every
projection is communication-free.  Only attention needs the peer's keys and
values: one AllGather of the packed (K^T, V_aug) block per attention, issued
right after the k/v projections and consumed after the q projection and the
core's own-key score blocks, so the collective hides under compute.  The
decoder's cross-attention K/V depend only on encoder memory and are
prefetched one layer ahead (ping-pong buffers).

A single SPMD program runs on all 8 cores.  Per-core differences are data:
  - token slices of codes / positional embeddings,
  - an index tile that makes the AllGather receive pick the PEER's rows,
  - exp-bias "gates" (0 or -1e9) that implement causal masking of the peer
    key blocks; keys live in a per-core permuted order (own tokens first)
    which attention is invariant to as long as masks agree.

Layout conventions (per core, P=128, TL=256 local tokens):
  x_nat[t]  [P, D]    fp32  natural tokens-on-partitions residual stream
  x_T       [P, ND*TL] bf16 transposed: chunk d cols [d*TL:(d+1)*TL]
  q_T       [P, ND*TL] bf16 rows = head dims (2 heads/chunk, 64 rows each)
  k_T       [P, ND*T]  bf16 chunk m cols: [own 256 | peer 256]
  v_aug     [P, 4*WA]  bf16 4 key slots x (per head 64 value + 64 ones cols);
                            slots 0,1 = own tiles, 2,3 = peer tiles
  attn_T    [P, ND*TL] bf16 context, transposed
  h_T       [P, NFF*TL] bf16 ffn hidden, transposed
"""

import os
import sys

for _p in ("/opt/trn_rl_repo", "/root/.axon_site/_ro/trn_rl_repo"):
    if os.path.isdir(_p) and _p not in sys.path:
        sys.path.insert(0, _p)

import numpy as np
import ml_dtypes

import concourse.bass as bass
import concourse.bacc as bacc
import concourse.tile as tile
from concourse import mybir
from concourse.bass import IndirectOffsetOnAxis
from concourse.masks import make_identity

BF16 = ml_dtypes.bfloat16
FP8NP = ml_dtypes.float8_e4m3fn
F32 = mybir.dt.float32
BF = mybir.dt.bfloat16
F8 = mybir.dt.float8e4
I32 = mybir.dt.int32
DR = mybir.MatmulPerfMode.DoubleRow
AF = mybir.ActivationFunctionType
ALU = mybir.AluOpType
AX = mybir.AxisListType

P = 128


class Cfg:
    def __init__(self, B, Q, T, D, H, V, L, FF, n_cores, flags=frozenset(),
                 fp8=False):
        self.B, self.Q, self.T, self.D, self.H, self.V, self.L, self.FF = \
            B, Q, T, D, H, V, L, FF
        self.n_cores = n_cores
        self.FP8 = fp8
        self.WSCALE = 64.0 if fp8 else 1.0   # host-side weight prescale
        self.SP = 2
        self.E = D // Q
        self.HD = D // H
        assert self.HD == 64, "head packing assumes head_dim 64"
        assert self.E == P, "per-quantizer embedding dim must be 128"
        self.SCALE = 1.0 / float(np.sqrt(self.HD))
        self.TL = T // self.SP          # local tokens
        self.NTL = self.TL // P         # local token tiles (2)
        self.NT = T // P                # all token tiles (4)
        self.ND = D // P                # 8
        self.NFF = FF // P              # 32
        self.WA = self.H * P            # v block width per key slot (2048)
        self.KSLOT = self.ND * P        # k block width per key slot (1024)
        self.AGW = self.NTL * (self.KSLOT + self.WA)  # half width (6144)
        self.flags = frozenset(flags)

    def kcol(self, s, m=0):
        """col of chunk m of key slot s in the combined kv tile."""
        return (s // 2) * self.AGW + (s % 2) * self.KSLOT + m * P

    def vcol(self, s):
        """col of the v_aug block of key slot s in the combined kv tile."""
        return (s // 2) * self.AGW + self.NTL * self.KSLOT + (s % 2) * self.WA

    def key(self):
        return (self.B, self.Q, self.T, self.D, self.H, self.V, self.L,
                self.FF, self.n_cores, self.FP8, tuple(sorted(self.flags)))


# --------------------------------------------------------------------------
# program builder
# --------------------------------------------------------------------------

def build_program(c: Cfg):
    nc = bacc.Bacc(None, target_bir_lowering=False, num_devices=c.n_cores)

    WDT = F8 if c.FP8 else BF

    def din(name, shape, dt=None):
        if dt is None:
            dt = WDT
        return nc.dram_tensor(name, shape, dt, kind="ExternalInput")

    x_emb_in = din("x_emb", [c.TL, c.D], F32)   # host-side embed + pos
    y_emb_in = din("y_emb", [c.TL, c.D], F32)
    rridx = din("rridx", [P, 1], I32)        # peer row indices in AG output
    gates = din("gates", [P, 4], F32)        # causal slot gates (0 / -1e9)

    w = {}
    NQKV = 3 * c.ND                          # 24 qkv chunks (q 0-7, k 8-15, v 16-23)
    for l in range(c.L):
        for nm, sh in [
                (f"e_qkv_{l}", [NQKV, P, c.D]), (f"e_out_{l}", [c.ND, P, c.D]),
                (f"e_ff1_{l}", [c.NFF, P, c.D]), (f"e_ff2_{l}", [c.NFF, P, c.D]),
                (f"d_sqkv_{l}", [NQKV, P, c.D]), (f"d_sout_{l}", [c.ND, P, c.D]),
                (f"d_cqkv_{l}", [NQKV, P, c.D]), (f"d_cout_{l}", [c.ND, P, c.D]),
                (f"d_ff1_{l}", [c.NFF, P, c.D]), (f"d_ff2_{l}", [c.NFF, P, c.D])]:
            w[nm] = din(nm, sh)
    w["head_t"] = din("head_t", [c.Q, P, c.V])

    opt = {}
    for nm in c.flags:
        if "_qkv_b_" in nm or "_sqkv_b_" in nm or "_cqkv_b_" in nm:
            opt[nm] = din(nm, [P, 2 * c.ND], F32)   # q,k bias cols per chunk
        elif "_ff1_b_" in nm:
            opt[nm] = din(nm, [P, c.NFF], F32)
        elif nm == "head_b":
            opt[nm] = din(nm, [P, c.Q * c.V], F32)
        else:
            opt[nm] = din(nm, [P, c.D], F32)

    logits = nc.dram_tensor("logits", [c.Q, c.TL, c.V], F32,
                            kind="ExternalOutput")

    groups = [[g * c.SP + i for i in range(c.SP)]
              for g in range(c.n_cores // c.SP)]

    with tile.TileContext(nc) as tc:
        _emit(nc, tc, c, x_emb_in, y_emb_in, rridx, gates,
              w, opt, logits, groups, WDT)
    nc.compile()
    return nc


def _emit(nc, tc, c, x_emb_in, y_emb_in, rridx, gates,
          w, opt, logits, groups, WDT):
    from contextlib import ExitStack
    es = ExitStack()
    pool = lambda name, bufs, space="SBUF": es.enter_context(
        tc.tile_pool(name=name, bufs=bufs, space=space))

    const = pool("const", 1)
    persist = pool("persist", 1)
    wpool = pool("wpool", 5)       # lhsT-style weight chunks (qkv/ff1)
    wk = pool("wk", 7)             # rhs-style weight chunks (v/out/ff2/head)
    act = pool("act", 3)
    scratch = pool("scratch", 2)
    dram = pool("dram", 4, space="DRAM")
    ps_big = pool("ps_big", 4, space="PSUM")    # 4 banks: accum for out/ff2/v
    ps_chunk = pool("ps_chunk", 2, space="PSUM")  # 2 banks: qkv/ff1/scores/tr
    ps_att = pool("ps_att", 2, space="PSUM")    # 2 banks: AV accum

    def wdma(i, out, in_, noscalar=False):
        """Weight-stream DMA spread over the two HWDGE queues.  The gpsimd
        SWDGE queue is reserved for AllGather receives so they never queue
        behind weight traffic."""
        engs = [nc.sync, nc.scalar]
        engs[i % 2].dma_start(out=out, in_=in_)

    ISC = 1.0 / 64.0 if c.FP8 else None   # inverse weight prescale

    def evac(dsl, psl, eng, bias=None):
        """PSUM -> SBUF copy with optional 1/WSCALE and bias fold."""
        if ISC is None:
            if bias is not None:
                nc.vector.tensor_scalar(out=dsl, in0=psl, scalar1=bias,
                                        scalar2=None, op0=ALU.add)
            elif eng == "s":
                nc.scalar.copy(out=dsl, in_=psl)
            else:
                nc.vector.tensor_copy(out=dsl, in_=psl)
        else:
            if bias is not None:
                nc.vector.tensor_scalar(out=dsl, in0=psl, scalar1=ISC,
                                        scalar2=bias, op0=ALU.mult,
                                        op1=ALU.add)
            elif eng == "s":
                nc.scalar.activation(dsl, psl, AF.Copy, scale=ISC)
            else:
                nc.vector.tensor_scalar_mul(dsl, psl, ISC)

    def big_tile():
        return ps_big.tile([P, 512], F32, tag="ps_big", name="psb")

    def chunk_tile():
        t = ps_chunk.tile([P, 512], F32, tag="ps_chunk", name="psc")
        return t

    def att_tile():
        return ps_att.tile([P, 512], F32, tag="ps_att", name="psa")

    # constants
    ident_f = const.tile([P, P], F32, name="ident_f")
    make_identity(nc, ident_f[:])
    causT = const.tile([P, P], F32, name="causT")
    nc.gpsimd.memset(causT[:], 0.0)
    # transposed causal: fill -1e9 where tk > tq  (iota = tq - tk >= 0 keeps)
    nc.gpsimd.affine_select(out=causT[:], in_=causT[:],
                            compare_op=ALU.is_ge, fill=-1e9, base=0,
                            pattern=[[1, P]], channel_multiplier=-1)
    eps_t = const.tile([P, 1], F32, name="eps_t")
    nc.vector.memset(eps_t[:], 1e-5)
    gates_sb = const.tile([P, 4], F32, name="gates_sb")
    nc.sync.dma_start(out=gates_sb[:], in_=gates[:])
    rridx_sb = const.tile([P, 1], I32, name="rridx_sb")
    nc.sync.dma_start(out=rridx_sb[:], in_=rridx[:])

    opt_sb = {}
    for nm in opt:
        shp = list(opt[nm].shape)
        t = const.tile(shp, F32, name=f"sb_{nm}")
        nc.sync.dma_start(out=t[:], in_=opt[nm][:])
        opt_sb[nm] = t

    # persistent activations
    x_nat = [persist.tile([P, c.D], F32, name=f"x_{t}", tag=f"x_{t}")
             for t in range(c.NTL)]
    y_emb = [persist.tile([P, c.D], F32, name=f"y_{t}", tag=f"y_{t}")
             for t in range(c.NTL)]
    ADT = F8 if c.FP8 else BF        # fat-matmul activation dtype
    x_T = persist.tile([P, c.ND * c.TL], ADT, name="x_T", tag="x_T")
    mem_T = persist.tile([P, c.ND * c.TL], ADT, name="mem_T", tag="mem_T")
    q_T = persist.tile([P, c.ND * c.TL], BF, name="q_T", tag="q_T")
    # combined (K^T, V_aug) tiles: [own half 6144 | peer half 6144]; each
    # half is [k slot, k slot, v slot, v slot] so the AG send/recv are single
    # contiguous row blocks.
    kv = persist.tile([P, 2 * c.AGW], BF, name="kv", tag="kv")
    ckv = [persist.tile([P, 2 * c.AGW], BF, name=f"ckv{i}", tag=f"ckv{i}")
           for i in range(2)]
    y_T0 = persist.tile([P, c.ND * c.TL], ADT, name="y_T0", tag="y_T0")
    attn_T = persist.tile([P, c.ND * c.TL], ADT, name="attn_T", tag="attn_T")
    h_T = persist.tile([P, c.NFF * c.TL], ADT, name="h_T", tag="h_T")

    # ones columns of the v_aug blocks (set once; projections/recv write the
    # value cols, and the AG transports the peer's ones columns verbatim)
    for kvt in [kv] + ckv:
        for s in range(c.NT):
            blk = kvt[:, c.vcol(s):c.vcol(s) + c.WA].rearrange(
                "p (h f) -> p h f", f=P)
            nc.vector.memset(blk[:, :, 64:128], 1.0)

    # ---------------- helpers ----------------
    def transpose_tile(dst, t, srct=None):
        """dst[:, d*TL + t*P : +P] = src[t][:, d*P:+P].T (fp32 cast)."""
        srcl = x_nat if srct is None else srct
        for d in range(c.ND):
            pt = ps_chunk.tile([P, P], F32, tag="ps_chunk", name="pt")
            nc.tensor.transpose(pt[:], srcl[t][:, d * P:(d + 1) * P],
                                ident_f[:])
            dsl = dst[:, d * c.TL + t * P: d * c.TL + t * P + P]
            if (t + d) % 2 == 0:
                nc.scalar.copy(out=dsl, in_=pt[:])
            else:
                nc.vector.tensor_copy(out=dsl, in_=pt[:])

    def transpose_x_into(dst):
        for t in range(c.NTL):
            transpose_tile(dst, t)

    def embed(src_dram, dst):
        for t in range(c.NTL):
            nc.sync.dma_start(out=dst[t][:],
                              in_=src_dram[t * P:(t + 1) * P, :])

    def ln_tile(t, sub, gname, bname):
        """x_nat[t] = LN(x_nat[t] + sub) * g + b (post-norm)."""
        xt = x_nat[t]
        s1 = scratch.tile([P, 1], F32, tag="lnstat", name="s1", bufs=8)
        s2 = scratch.tile([P, 1], F32, tag="lnstat", name="s2", bufs=8)
        sq = scratch.tile([P, c.D], BF, tag="lnsq", name="sq")
        nc.vector.tensor_tensor(out=xt[:], in0=xt[:], in1=sub[:], op=ALU.add)
        nc.vector.reduce_sum(out=s1[:], in_=xt[:], axis=AX.X)
        nc.scalar.activation(sq[:], xt[:], AF.Square, accum_out=s2[:])
        mean = scratch.tile([P, 1], F32, tag="lnstat", name="mean", bufs=8)
        var = scratch.tile([P, 1], F32, tag="lnstat", name="var", bufs=8)
        m2 = scratch.tile([P, 1], F32, tag="lnstat", name="m2", bufs=8)
        nc.vector.tensor_scalar_mul(mean[:], s1[:], 1.0 / c.D)
        nc.vector.tensor_scalar_mul(var[:], s2[:], 1.0 / c.D)
        nc.vector.tensor_tensor(out=m2[:], in0=mean[:], in1=mean[:],
                                op=ALU.mult)
        nc.vector.tensor_tensor(out=var[:], in0=var[:], in1=m2[:],
                                op=ALU.subtract)
        rstd = scratch.tile([P, 1], F32, tag="lnstat", name="rstd", bufs=8)
        nc.scalar.activation(rstd[:], var[:], AF.Sqrt, bias=eps_t[:])
        nc.vector.reciprocal(rstd[:], rstd[:])
        nc.vector.tensor_scalar(out=xt[:], in0=xt[:], scalar1=mean[:],
                                scalar2=rstd[:], op0=ALU.subtract,
                                op1=ALU.mult)
        if gname in opt_sb:
            nc.vector.tensor_tensor(out=xt[:], in0=xt[:],
                                    in1=opt_sb[gname][:], op=ALU.mult)
        if bname in opt_sb:
            nc.vector.tensor_tensor(out=xt[:], in0=xt[:],
                                    in1=opt_sb[bname][:], op=ALU.add)

    def proj_qk(wname, bname, src_T, dst, kind):
        """kind='q': chunks 0-7 -> dst[:, m*TL cols]; kind='k': chunks 8-15
        -> the own key slots of the combined kv tile `dst`."""
        cbase = 0 if kind == "q" else c.ND
        for m in range(c.ND):
            wt = wpool.tile([P, c.D], WDT, tag="wqkv", name="wt")
            wdma(m, wt[:], w[wname][cbase + m], noscalar=True)
            ps = chunk_tile()[:, 0:c.TL]
            if c.FP8:
                wt3 = wt[:].rearrange("p (k m) -> p k m", m=P)
                x3 = src_T[:].rearrange("p (k t) -> p k t", t=c.TL)
                for j in range(c.ND // 2):
                    nc.tensor.matmul(ps[:], wt3[:, 2 * j:2 * j + 2, :],
                                     x3[:, 2 * j:2 * j + 2, :],
                                     start=(j == 0), stop=(j == c.ND // 2 - 1),
                                     perf_mode=DR)
            else:
                for k in range(c.ND):
                    nc.tensor.matmul(ps[:], wt[:, k * P:(k + 1) * P],
                                     src_T[:, k * c.TL:(k + 1) * c.TL],
                                     start=(k == 0), stop=(k == c.ND - 1))
            if bname in opt_sb:
                bcol = m if kind == "q" else c.ND + m
                bias = opt_sb[bname][:, bcol:bcol + 1]
            else:
                bias = None
            if kind == "q":
                evac(dst[:, m * c.TL:(m + 1) * c.TL], ps[:],
                     "v" if m % 2 == 0 else "s", bias)
            else:
                for tl in range(c.NTL):
                    dsl = dst[:, c.kcol(tl, m):c.kcol(tl, m) + P]
                    psl = ps[:, tl * P:(tl + 1) * P]
                    evac(dsl, psl, "v", bias)

    def proj_v(wname, src_T, dst_aug):
        """chunks 16-23 (rhs layout): natural v for own tiles -> dst_aug
        slots 0..NTL-1 data columns."""
        NB = c.D // 512
        pss = [[big_tile() for n in range(NB)] for t in range(c.NTL)]
        if c.FP8:
            src3 = src_T[:].rearrange("p (k t) -> p k t", t=c.TL)
            for j in range(c.ND // 2):
                wt = wk.tile([P, 2 * c.D], F8, tag="wv", name="wt")
                wt3 = wt[:].rearrange("p (k d) -> p k d", d=c.D)
                wdma(j, wt3,
                     w[wname][2 * c.ND + 2 * j:2 * c.ND + 2 * j + 2]
                     .rearrange("k p d -> p k d"))
                for t in range(c.NTL):
                    for n in range(NB):
                        nc.tensor.matmul(
                            pss[t][n][:],
                            src3[:, 2 * j:2 * j + 2, t * P:t * P + P],
                            wt3[:, :, n * 512:(n + 1) * 512],
                            start=(j == 0), stop=(j == c.ND // 2 - 1),
                            perf_mode=DR)
        else:
            for k in range(c.ND):
                wt = wk.tile([P, c.D], BF, tag="wv", name="wt")
                wdma(k, wt[:], w[wname][2 * c.ND + k], noscalar=True)
                for t in range(c.NTL):
                    for n in range(NB):
                        nc.tensor.matmul(
                            pss[t][n][:],
                            src_T[:, k * c.TL + t * P: k * c.TL + t * P + P],
                            wt[:, n * 512:(n + 1) * 512],
                            start=(k == 0), stop=(k == c.ND - 1))
        for t in range(c.NTL):
            blk = dst_aug[:, c.vcol(t):c.vcol(t) + c.WA].rearrange(
                "p (h f) -> p h f", f=P)
            for n in range(NB):
                # 512 cols = 8 heads' worth of 64-wide value blocks
                psv = pss[t][n][:].rearrange("p (h f) -> p h f", f=64)
                dstb = blk[:, n * 8:(n + 1) * 8, 0:64]
                if c.FP8:
                    nc.scalar.activation(dstb, psv, AF.Copy, scale=1.0 / 64.0)
                else:
                    nc.scalar.copy(out=dstb, in_=psv)

    KR = c.NTL * c.KSLOT            # k region width per half (2048)
    VR = c.AGW - KR

    def ag_start(kv_dst, lo, hi, tag):
        """Send + AllGather-trigger for kv_dst[:, lo:hi].  The receive is a
        SEPARATE call so triggers never queue behind earlier receives on the
        in-order gpsimd stream."""
        agin = dram.tile([P, hi - lo], BF, tag=f"agin{tag}", name="agin")
        agout = dram.tile([c.SP * P, hi - lo], BF, tag=f"agout{tag}",
                          name="agout")
        with tc.high_priority():
            nc.sync.dma_start(out=agin[:], in_=kv_dst[:, lo:hi])
        nc.gpsimd.collective_compute(
            "AllGather", ALU.bypass, replica_groups=groups,
            ins=[agin[:].opt()], outs=[agout[:].opt()])
        return (agout, kv_dst, c.AGW + lo)

    def ag_recv(h):
        agout, kv_dst, dst0 = h
        n = agout.shape[-1]
        nc.gpsimd.indirect_dma_start(
            out=kv_dst[:, dst0:dst0 + n],
            out_offset=None,
            in_=agout[:],
            in_offset=IndirectOffsetOnAxis(ap=rridx_sb[:, :1], axis=0))

    def ag_k_start(kv_dst):
        return ag_start(kv_dst, 0, KR, "k")

    def ag_v_start(kv_dst):
        return ag_start(kv_dst, KR, c.AGW, "v")

    def attention(kvt, mode):
        """q_T x kvt -> attn_T.  mode: 'full' (all 4 slots, no mask) or
        'causal' (slot structure for decoder self-attention)."""
        for h in range(c.H):
            m = h // 2
            po = 64 * (h % 2)
            # score slot pairs share one PSUM bank + one exp where possible:
            # pair block cols [s%2 * TL : +TL]; in causal mode slot 1 only
            # has q cols [P:TL] (placed at [TL+P:2TL]) and slots 0/1 get the
            # diagonal causT mask; slots 2/3 share one gate bias.
            at = []
            for pi, (s0, s1) in enumerate([(0, 1), (2, 3)]):
                pss = big_tile()
                a = act.tile([P, 2 * c.TL], BF, tag=f"ATp{pi}", name="at",
                             bufs=3)
                qr = []
                for si, s in enumerate((s0, s1)):
                    q0 = P if (mode == "causal" and s == 1) else 0
                    base = si * c.TL
                    nc.tensor.matmul(
                        pss[:, base + q0: base + c.TL],
                        kvt[po:po + 64, c.kcol(s, m):c.kcol(s, m) + P],
                        q_T[po:po + 64, m * c.TL + q0: m * c.TL + c.TL],
                        start=True, stop=True)
                    if mode == "causal" and s < 2:
                        # diagonal block: q columns s*P of this slot
                        d0 = base + s * P
                        nc.vector.tensor_tensor(
                            out=pss[:, d0:d0 + P], in0=pss[:, d0:d0 + P],
                            in1=causT[:], op=ALU.add)
                    qr.append(q0)
                if mode == "causal" and pi == 1:
                    nc.scalar.activation(a[:], pss[:], AF.Exp, scale=c.SCALE,
                                         bias=gates_sb[:, s0:s0 + 1])
                elif mode == "causal":
                    # two exps: slot0 full, slot1 partial (skip the dead gap)
                    nc.scalar.activation(a[:, 0:c.TL], pss[:, 0:c.TL],
                                         AF.Exp, scale=c.SCALE)
                    nc.scalar.activation(a[:, c.TL + P:2 * c.TL],
                                         pss[:, c.TL + P:2 * c.TL],
                                         AF.Exp, scale=c.SCALE)
                else:
                    nc.scalar.activation(a[:], pss[:], AF.Exp, scale=c.SCALE)
                at.append((s0, a, 0, qr[0]))
                at.append((s1, a, c.TL, qr[1]))
            ps_o = att_tile()[:, 0:c.TL]
            for i, (s, a, base, q0) in enumerate(at):
                nc.tensor.matmul(
                    ps_o[:, q0:c.TL],
                    kvt[:, c.vcol(s) + h * P: c.vcol(s) + (h + 1) * P],
                    a[:, base + q0: base + c.TL],
                    start=(i == 0), stop=(i == len(at) - 1))
            rden = scratch.tile([64, c.TL], F32, tag="rden", name="rden",
                                bufs=2)
            nc.vector.reciprocal(rden[:], ps_o[64:128, :])
            nc.vector.tensor_tensor(
                out=attn_T[po:po + 64, m * c.TL:(m + 1) * c.TL],
                in0=ps_o[0:64, :], in1=rden[:], op=ALU.mult)

    def mm_to_natural(src_T, nk, wname, bname, noscalar=False):
        """[TL, D] = src_T.T @ W (k-chunk streaming, PSUM accumulate),
        returned as per-t bf16 [P, D] tiles."""
        NB = c.D // 512
        pss = [[big_tile() for n in range(NB)] for t in range(c.NTL)]
        if c.FP8:
            src3 = src_T[:].rearrange("p (k t) -> p k t", t=c.TL)
            for j in range(nk // 2):
                wt = wk.tile([P, 2 * c.D], F8, tag="wnat", name="wt")
                wt3 = wt[:].rearrange("p (k d) -> p k d", d=c.D)
                wdma(j, wt3,
                     w[wname][2 * j:2 * j + 2].rearrange("k p d -> p k d"))
                for t in range(c.NTL):
                    for n in range(NB):
                        nc.tensor.matmul(
                            pss[t][n][:],
                            src3[:, 2 * j:2 * j + 2, t * P:t * P + P],
                            wt3[:, :, n * 512:(n + 1) * 512],
                            start=(j == 0), stop=(j == nk // 2 - 1),
                            perf_mode=DR)
        else:
            for k in range(nk):
                wt = wk.tile([P, c.D], BF, tag="wnat", name="wt")
                wdma(k, wt[:], w[wname][k], noscalar=noscalar)
                for t in range(c.NTL):
                    for n in range(NB):
                        nc.tensor.matmul(
                            pss[t][n][:],
                            src_T[:, k * c.TL + t * P: k * c.TL + t * P + P],
                            wt[:, n * 512:(n + 1) * 512],
                            start=(k == 0), stop=(k == nk - 1))
        parts = []
        for t in range(c.NTL):
            sb = scratch.tile([P, c.D], BF, tag="oproj", name="sb", bufs=3)
            for n in range(NB):
                evac(sb[:, n * 512:(n + 1) * 512], pss[t][n][:],
                     "s" if (t + n) % 2 == 0 else "v")
            if bname in opt_sb:
                nc.vector.tensor_tensor(out=sb[:], in0=sb[:],
                                        in1=opt_sb[bname][:], op=ALU.add)
            parts.append(sb)
        return parts

    def ffn(w1name, b1name, w2name, b2name):
        x3 = x_T[:].rearrange("p (k t) -> p k t", t=c.TL)
        for mchunk in range(c.NFF):
            wt = wpool.tile([P, c.D], WDT, tag="wff1", name="wt")
            wdma(mchunk, wt[:], w[w1name][mchunk])
            ps = chunk_tile()[:, 0:c.TL]
            if c.FP8:
                wt3 = wt[:].rearrange("p (k m) -> p k m", m=P)
                for j in range(c.ND // 2):
                    nc.tensor.matmul(ps[:], wt3[:, 2 * j:2 * j + 2, :],
                                     x3[:, 2 * j:2 * j + 2, :],
                                     start=(j == 0), stop=(j == c.ND // 2 - 1),
                                     perf_mode=DR)
            else:
                for k in range(c.ND):
                    nc.tensor.matmul(ps[:], wt[:, k * P:(k + 1) * P],
                                     x_T[:, k * c.TL:(k + 1) * c.TL],
                                     start=(k == 0), stop=(k == c.ND - 1))
            dsl = h_T[:, mchunk * c.TL:(mchunk + 1) * c.TL]
            kw = {"scale": 1.0 / 64.0} if c.FP8 else {}
            if b1name in opt_sb:
                nc.scalar.activation(dsl, ps[:], AF.Relu,
                                     bias=opt_sb[b1name][:, mchunk:mchunk + 1],
                                     **kw)
            else:
                nc.scalar.activation(dsl, ps[:], AF.Relu, **kw)
        return mm_to_natural(h_T, c.NFF, w2name, b2name)

    def ag_warm(dep_tile):
        """Tiny AllGather that wakes the cc stream ahead of a real AG;
        dep_tile pins its position in the schedule."""
        n = dep_tile.shape[-1]
        win = dram.tile([P, n], dep_tile.dtype, tag="warmin", name="win")
        wout = dram.tile([c.SP * P, n], dep_tile.dtype, tag="warmout",
                         name="wout")
        nc.sync.dma_start(out=win[:], in_=dep_tile)
        nc.gpsimd.collective_compute(
            "AllGather", ALU.bypass, replica_groups=groups,
            ins=[win[:].opt()], outs=[wout[:].opt()])

    def cross_kv_proj(l):
        buf = l % 2
        proj_qk(f"d_cqkv_{l}", f"d_cqkv_b_{l}", mem_T, ckv[buf], "k")
        proj_v(f"d_cqkv_{l}", mem_T, ckv[buf])

    def cross_kv_prefetch(l):
        """Project + AllGather cross-attention K/V for decoder layer l."""
        cross_kv_proj(l)
        hk = ag_k_start(ckv[l % 2])
        hv = ag_v_start(ckv[l % 2])
        ag_recv(hk)
        ag_recv(hv)

    # ---------------- encoder ----------------
    ag_warm(gates_sb[:, 0:4])
    embed(x_emb_in, x_nat)
    embed(y_emb_in, y_emb)
    transpose_x_into(x_T)
    for l in range(c.L):
        proj_qk(f"e_qkv_{l}", f"e_qkv_b_{l}", x_T, kv, "k")
        if l > 0:
            ag_warm(kv[:, 0:8])
        hk = ag_k_start(kv)
        proj_v(f"e_qkv_{l}", x_T, kv)
        hv = ag_v_start(kv)
        hek = hev = None
        if l == 0:
            # decoder layer-0 self K/V depend only on the target embedding:
            # project + gather them now, into the idle ckv[1] buffer, so the
            # PE work and cc traffic fill the encoder warm-up stalls
            for t in range(c.NTL):
                transpose_tile(y_T0, t, srct=y_emb)
            proj_qk("d_sqkv_0", "d_sqkv_b_0", y_T0, ckv[1], "k")
            hek = ag_k_start(ckv[1])
            proj_v("d_sqkv_0", y_T0, ckv[1])
            hev = ag_v_start(ckv[1])
        ag_recv(hk)
        ag_recv(hv)
        if hek is not None:
            ag_recv(hek)
            ag_recv(hev)
        proj_qk(f"e_qkv_{l}", f"e_qkv_b_{l}", x_T, q_T, "q")
        attention(kv, "full")
        parts = mm_to_natural(attn_T, c.ND, f"e_out_{l}", f"e_out_b_{l}",
                              noscalar=True)
        for t in range(c.NTL):
            ln_tile(t, parts[t], f"e_ln1_w_{l}", f"e_ln1_b_{l}")
            transpose_tile(x_T, t)
        parts = ffn(f"e_ff1_{l}", f"e_ff1_b_{l}", f"e_ff2_{l}", f"e_ff2_b_{l}")
        last = l == c.L - 1
        for t in range(c.NTL):
            ln_tile(t, parts[t], f"e_ln2_w_{l}", f"e_ln2_b_{l}")
            transpose_tile(mem_T if last else x_T, t)

    # cross K/V for decoder layer 0 (hides under decoder embed + self attn)
    cross_kv_prefetch(0)

    # ---------------- decoder ----------------
    for t in range(c.NTL):
        nc.vector.tensor_copy(out=x_nat[t][:], in_=y_emb[t][:])
        transpose_tile(x_T, t)
    for l in range(c.L):
        if l > 0:
            proj_qk(f"d_sqkv_{l}", f"d_sqkv_b_{l}", x_T, kv, "k")
            ag_warm(kv[:, 0:8])
            hk = ag_k_start(kv)
            proj_v(f"d_sqkv_{l}", x_T, kv)
            hv = ag_v_start(kv)
            hck = hcv = None
            if l + 1 < c.L:
                cross_kv_proj(l + 1)
                hck = ag_k_start(ckv[(l + 1) % 2])
                hcv = ag_v_start(ckv[(l + 1) % 2])
            ag_recv(hk)
            ag_recv(hv)
            if hck is not None:
                ag_recv(hck)
                ag_recv(hcv)
            proj_qk(f"d_sqkv_{l}", f"d_sqkv_b_{l}", x_T, q_T, "q")
            attention(kv, "causal")
        else:
            # layer-0 self K/V were projected + gathered during the encoder;
            # consume them from ckv[1] BEFORE cross_kv_proj(1) recycles it
            ag_warm(x_T[:, 0:8])
            proj_qk("d_sqkv_0", "d_sqkv_b_0", x_T, q_T, "q")
            attention(ckv[1], "causal")
            cross_kv_proj(1)
            hck = ag_k_start(ckv[1])
            hcv = ag_v_start(ckv[1])
            ag_recv(hck)
            ag_recv(hcv)
        parts = mm_to_natural(attn_T, c.ND, f"d_sout_{l}", f"d_sout_b_{l}",
                              noscalar=True)
        for t in range(c.NTL):
            ln_tile(t, parts[t], f"d_ln1_w_{l}", f"d_ln1_b_{l}")
            transpose_tile(x_T, t)
        proj_qk(f"d_cqkv_{l}", f"d_cqkv_b_{l}", x_T, q_T, "q")
        attention(ckv[l % 2], "full")
        parts = mm_to_natural(attn_T, c.ND, f"d_cout_{l}", f"d_cout_b_{l}",
                              noscalar=True)
        for t in range(c.NTL):
            ln_tile(t, parts[t], f"d_ln2_w_{l}", f"d_ln2_b_{l}")
            transpose_tile(x_T, t)
        parts = ffn(f"d_ff1_{l}", f"d_ff1_b_{l}", f"d_ff2_{l}", f"d_ff2_b_{l}")
        for t in range(c.NTL):
            ln_tile(t, parts[t], f"d_ln3_w_{l}", f"d_ln3_b_{l}")
            transpose_tile(x_T, t)

    # ---------------- output head ----------------
    NBV = c.V // 512
    for j in range(c.Q):
        hw = wk.tile([P, c.V], WDT, tag="whead", name="hw", bufs=2)
        wdma(j, hw[:], w["head_t"][j])
        for t in range(c.NTL):
            sb = scratch.tile([P, c.V], F32, tag="lgt", name="sb", bufs=3)
            for n in range(NBV):
                ps = big_tile()
                nc.tensor.matmul(
                    ps[:], x_T[:, j * c.TL + t * P: j * c.TL + t * P + P],
                    hw[:, n * 512:(n + 1) * 512], start=True, stop=True)
                evac(sb[:, n * 512:(n + 1) * 512], ps[:], "s")
            if "head_b" in opt_sb:
                nc.vector.tensor_tensor(
                    out=sb[:], in0=sb[:],
                    in1=opt_sb["head_b"][:, j * c.V:(j + 1) * c.V],
                    op=ALU.add)
            nc.sync.dma_start(out=logits[j, t * P:(t + 1) * P, :], in_=sb[:])

    es.close()


# --------------------------------------------------------------------------
# host side
# --------------------------------------------------------------------------

_PROG_CACHE = {}


def parse_cfg(inputs, n_cores=8, fp8=None):
    B, Q, T = inputs["input_codes"].shape
    _, V, E = np.asarray(inputs["tok_emb"]).shape
    L, _, D = np.asarray(inputs["e_qkv_w"]).shape
    FF = np.asarray(inputs["e_ff1_w"]).shape[1]
    H = D // 64
    flags = set()
    for l in range(L):
        for knm in ["e_qkv_b", "d_sqkv_b", "d_cqkv_b", "e_ff1_b", "d_ff1_b",
                    "e_out_b", "e_ff2_b", "d_sout_b", "d_cout_b", "d_ff2_b"]:
            if np.any(np.asarray(inputs[knm])[l]):
                flags.add(f"{knm}_{l}")
        for ln in ["e_ln1", "e_ln2", "d_ln1", "d_ln2", "d_ln3"]:
            if not np.all(np.asarray(inputs[ln + "_w"])[l] == 1.0):
                flags.add(f"{ln}_w_{l}")
            if np.any(np.asarray(inputs[ln + "_b"])[l]):
                flags.add(f"{ln}_b_{l}")
    if np.any(np.asarray(inputs["head_b"])):
        flags.add("head_b")
    if fp8 is None:
        fp8 = os.environ.get("BASS_S2S_FP8", "0") == "1"
    # v-bias unsupported in-kernel; fall back assertion
    for l in range(L):
        for nm in ["e_qkv_b", "d_sqkv_b", "d_cqkv_b"]:
            vb = np.asarray(inputs[nm])[l][2 * D:3 * D]
            assert not np.any(vb), "nonzero v bias not supported"
    return Cfg(B, Q, T, D, H, V, L, FF, n_cores, flags, fp8=fp8)


def _lhsT_chunks(wm, D):
    """[M, D] row-major weight -> [M//128, 128(p=in%128), ...] lhsT chunk
    layout: chunk c element [p, nd*128 + m] = wm[c*128 + m, nd*128 + p]."""
    M = wm.shape[0]
    nd = D // P
    out = np.empty((M // P, P, D), np.float32)
    for cc in range(M // P):
        wc = wm[cc * P:(cc + 1) * P, :]          # [128 m, D in]
        out[cc] = wc.T.reshape(nd, P, P).transpose(1, 0, 2).reshape(P, D)
    return out


def _rhs_chunks(wm, D_out):
    """[D_out, K] row-major weight -> [K//128, 128(p=k%128), D_out] rhs
    chunk layout: chunk k element [p, n] = wm[n, k*128 + p]."""
    K = wm.shape[1]
    return np.ascontiguousarray(
        wm.T.reshape(K // P, P, D_out))


def build_inmaps(inputs, c: Cfg):
    g = lambda nm: np.asarray(inputs[nm], np.float32)
    if c.FP8:
        def bf(a):
            a = np.ascontiguousarray(a, dtype=np.float32) * c.WSCALE
            return np.clip(a, -240.0, 240.0).astype(FP8NP)
    else:
        bf = lambda a: np.ascontiguousarray(a, dtype=np.float32).astype(BF16)

    tok = np.asarray(inputs["tok_emb"], np.float32)
    posf = np.ascontiguousarray(g("pos_emb")[0, :c.T, :])
    head_w = g("head_w")

    common = {}
    head_t = np.stack([head_w[q].T for q in range(c.Q)])    # [Q, E, V]
    common["head_t"] = bf(head_t)
    if "head_b" in c.flags:
        hb = g("head_b").reshape(-1)
        common["head_b"] = np.broadcast_to(hb, (P, c.Q * c.V)).copy()

    for pre, wq, wo in [("e_qkv", "e_qkv_w", None), ("e_out", None, "e_out_w"),
                        ("d_sqkv", "d_sqkv_w", None),
                        ("d_sout", None, "d_sout_w"),
                        ("d_cqkv", "d_cqkv_w", None),
                        ("d_cout", None, "d_cout_w")]:
        for l in range(c.L):
            if wq is not None:
                qkv = g(wq)[l]                    # [3D, D]
                qk = _lhsT_chunks(qkv[0:2 * c.D], c.D)      # q,k chunks
                vv = _rhs_chunks(qkv[2 * c.D:3 * c.D], c.D)
                common[f"{pre}_{l}"] = bf(np.concatenate([qk, vv], axis=0))
            else:
                wo_l = g(wo)[l]                   # [D, D] rows = out dim
                common[f"{pre}_{l}"] = bf(_rhs_chunks(wo_l, c.D))
    for l in range(c.L):
        common[f"e_ff1_{l}"] = bf(_lhsT_chunks(g("e_ff1_w")[l], c.D))
        common[f"d_ff1_{l}"] = bf(_lhsT_chunks(g("d_ff1_w")[l], c.D))
        common[f"e_ff2_{l}"] = bf(_rhs_chunks(g("e_ff2_w")[l], c.D))
        common[f"d_ff2_{l}"] = bf(_rhs_chunks(g("d_ff2_w")[l], c.D))

    # optional biases
    for l in range(c.L):
        for knm in ["e_qkv_b", "d_sqkv_b", "d_cqkv_b"]:
            if f"{knm}_{l}" in c.flags:
                b = g(knm)[l][0:2 * c.D]           # q,k bias only
                common[f"{knm}_{l}"] = np.ascontiguousarray(
                    b.reshape(2 * c.ND, P).T)
        for knm in ["e_ff1_b", "d_ff1_b"]:
            if f"{knm}_{l}" in c.flags:
                common[f"{knm}_{l}"] = np.ascontiguousarray(
                    g(knm)[l].reshape(c.NFF, P).T)
        for knm in ["e_out_b", "e_ff2_b", "d_sout_b", "d_cout_b", "d_ff2_b"]:
            if f"{knm}_{l}" in c.flags:
                common[f"{knm}_{l}"] = np.broadcast_to(
                    g(knm)[l], (P, c.D)).copy()
        for ln in ["e_ln1", "e_ln2", "d_ln1", "d_ln2", "d_ln3"]:
            for sfx in ["w", "b"]:
                if f"{ln}_{sfx}_{l}" in c.flags:
                    common[f"{ln}_{sfx}_{l}"] = np.broadcast_to(
                        g(f"{ln}_{sfx}")[l], (P, c.D)).copy()

    codes_in = np.asarray(inputs["input_codes"], np.int32)
    codes_tgt = np.asarray(inputs["target_codes"], np.int32)

    def embed_host(codes_bqt):
        # [Q, T] codes -> [T, D] embedding (concat per-quantizer) + pos
        e = np.concatenate([tok[q][codes_bqt[q]] for q in range(c.Q)],
                           axis=-1)
        return e + posf

    emb_in = [embed_host(codes_in[b]) for b in range(c.B)]
    emb_tgt = [embed_host(codes_tgt[b]) for b in range(c.B)]
    in_maps = []
    for core in range(c.n_cores):
        b, h = core // c.SP, core % c.SP
        m = dict(common)
        sl = slice(h * c.TL, (h + 1) * c.TL)
        m["x_emb"] = np.ascontiguousarray(emb_in[b % c.B][sl])
        m["y_emb"] = np.ascontiguousarray(emb_tgt[b % c.B][sl])
        m["rridx"] = ((1 - h) * P + np.arange(P, dtype=np.int32)
                      ).reshape(P, 1)
        gate = np.zeros((P, 4), np.float32)
        if h == 0:
            gate[:, 2] = -1e9
            gate[:, 3] = -1e9
        m["gates"] = gate
        in_maps.append(m)
    return in_maps


def postprocess(results, c: Cfg):
    out = np.empty((c.B, c.T, c.Q, c.V), np.float32)
    for b in range(c.B):
        for h in range(c.SP):
            r = results[b * c.SP + h]["logits"]      # [Q, TL, V]
            out[b, h * c.TL:(h + 1) * c.TL] = r.transpose(1, 0, 2)
    return out


def run(inputs, trace=False):
    from concourse.bass_utils import run_bass_kernel_spmd
    c = parse_cfg(inputs)
    key = c.key()
    if key not in _PROG_CACHE:
        _PROG_CACHE[key] = build_program(c)
    nc = _PROG_CACHE[key]
    in_maps = build_inmaps(inputs, c)
    res = run_bass_kernel_spmd(nc, in_maps, list(range(c.n_cores)),
                               trace=trace)
    return postprocess(res.results, c), res


def kernel(**inputs):
    out, _ = run(inputs, trace=False)
    return out


# revision 37
# speedup vs baseline: 1.0183x; 1.0183x over previous
"""Trainium2 Bass kernel for an encoder-decoder (S2S) transformer.

Distribution: 8 NeuronCores = 4 data-parallel groups (batch B=4) x 2-way
SEQUENCE-parallel within each pair.  Each core owns 256 tokens (2 tiles of
128) of one batch element at full model width, so layernorm, FFN and every
projection is communication-free.  Only attention needs the peer's keys and
values: one AllGather of the packed (K^T, V_aug) block per attention, issued
right after the k/v projections and consumed after the q projection and the
core's own-key score blocks, so the collective hides under compute.  The
decoder's cross-attention K/V depend only on encoder memory and are
prefetched one layer ahead (ping-pong buffers).

A single SPMD program runs on all 8 cores.  Per-core differences are data:
  - token slices of codes / positional embeddings,
  - an index tile that makes the AllGather receive pick the PEER's rows,
  - exp-bias "gates" (0 or -1e9) that implement causal masking of the peer
    key blocks; keys live in a per-core permuted order (own tokens first)
    which attention is invariant to as long as masks agree.

Layout conventions (per core, P=128, TL=256 local tokens):
  x_nat[t]  [P, D]    fp32  natural tokens-on-partitions residual stream
  x_T       [P, ND*TL] bf16 transposed: chunk d cols [d*TL:(d+1)*TL]
  q_T       [P, ND*TL] bf16 rows = head dims (2 heads/chunk, 64 rows each)
  k_T       [P, ND*T]  bf16 chunk m cols: [own 256 | peer 256]
  v_aug     [P, 4*WA]  bf16 4 key slots x (per head 64 value + 64 ones cols);
                            slots 0,1 = own tiles, 2,3 = peer tiles
  attn_T    [P, ND*TL] bf16 context, transposed
  h_T       [P, NFF*TL] bf16 ffn hidden, transposed
"""

import os
import sys

for _p in ("/opt/trn_rl_repo", "/root/.axon_site/_ro/trn_rl_repo"):
    if os.path.isdir(_p) and _p not in sys.path:
        sys.path.insert(0, _p)

import numpy as np
import ml_dtypes

import concourse.bass as bass
import concourse.bacc as bacc
import concourse.tile as tile
from concourse import mybir
from concourse.bass import IndirectOffsetOnAxis
from concourse.masks import make_identity

BF16 = ml_dtypes.bfloat16
FP8NP = ml_dtypes.float8_e4m3fn
F32 = mybir.dt.float32
BF = mybir.dt.bfloat16
F8 = mybir.dt.float8e4
I32 = mybir.dt.int32
DR = mybir.MatmulPerfMode.DoubleRow
AF = mybir.ActivationFunctionType
ALU = mybir.AluOpType
AX = mybir.AxisListType

P = 128


class Cfg:
    def __init__(self, B, Q, T, D, H, V, L, FF, n_cores, flags=frozenset(),
                 fp8=False):
        self.B, self.Q, self.T, self.D, self.H, self.V, self.L, self.FF = \
            B, Q, T, D, H, V, L, FF
        self.n_cores = n_cores
        self.FP8 = fp8
        self.WSCALE = 64.0 if fp8 else 1.0   # host-side weight prescale
        self.SP = 2
        self.E = D // Q
        self.HD = D // H
        assert self.HD == 64, "head packing assumes head_dim 64"
        assert self.E == P, "per-quantizer embedding dim must be 128"
        self.SCALE = 1.0 / float(np.sqrt(self.HD))
        self.TL = T // self.SP          # local tokens
        self.NTL = self.TL // P         # local token tiles (2)
        self.NT = T // P                # all token tiles (4)
        self.ND = D // P                # 8
        self.NFF = FF // P              # 32
        self.WA = self.H * P            # v block width per key slot (2048)
        self.KSLOT = self.ND * P        # k block width per key slot (1024)
        self.AGW = self.NTL * (self.KSLOT + self.WA)  # half width (6144)
        self.flags = frozenset(flags)

    def kcol(self, s, m=0):
        """col of chunk m of key slot s in the combined kv tile."""
        return (s // 2) * self.AGW + (s % 2) * self.KSLOT + m * P

    def vcol(self, s):
        """col of the v_aug block of key slot s in the combined kv tile."""
        return (s // 2) * self.AGW + self.NTL * self.KSLOT + (s % 2) * self.WA

    def key(self):
        return (self.B, self.Q, self.T, self.D, self.H, self.V, self.L,
                self.FF, self.n_cores, self.FP8, tuple(sorted(self.flags)))


# --------------------------------------------------------------------------
# program builder
# --------------------------------------------------------------------------

def build_program(c: Cfg):
    nc = bacc.Bacc(None, target_bir_lowering=False, num_devices=c.n_cores)

    WDT = F8 if c.FP8 else BF

    def din(name, shape, dt=None):
        if dt is None:
            dt = WDT
        return nc.dram_tensor(name, shape, dt, kind="ExternalInput")

    x_emb_in = din("x_emb", [c.TL, c.D], F32)   # host-side embed + pos
    y_emb_in = din("y_emb", [c.TL, c.D], F32)
    rridx = din("rridx", [P, 1], I32)        # peer row indices in AG output
    gates = din("gates", [P, 4], F32)        # causal slot gates (0 / -1e9)

    w = {}
    NQKV = 3 * c.ND                          # 24 qkv chunks (q 0-7, k 8-15, v 16-23)
    for l in range(c.L):
        for nm, sh in [
                (f"e_qkv_{l}", [NQKV, P, c.D]), (f"e_out_{l}", [c.ND, P, c.D]),
                (f"e_ff1_{l}", [c.NFF, P, c.D]), (f"e_ff2_{l}", [c.NFF, P, c.D]),
                (f"d_sqkv_{l}", [NQKV, P, c.D]), (f"d_sout_{l}", [c.ND, P, c.D]),
                (f"d_cqkv_{l}", [NQKV, P, c.D]), (f"d_cout_{l}", [c.ND, P, c.D]),
                (f"d_ff1_{l}", [c.NFF, P, c.D]), (f"d_ff2_{l}", [c.NFF, P, c.D])]:
            w[nm] = din(nm, sh)
    w["head_t"] = din("head_t", [c.Q, P, c.V])

    opt = {}
    for nm in c.flags:
        if "_qkv_b_" in nm or "_sqkv_b_" in nm or "_cqkv_b_" in nm:
            opt[nm] = din(nm, [P, 2 * c.ND], F32)   # q,k bias cols per chunk
        elif "_ff1_b_" in nm:
            opt[nm] = din(nm, [P, c.NFF], F32)
        elif nm == "head_b":
            opt[nm] = din(nm, [P, c.Q * c.V], F32)
        else:
            opt[nm] = din(nm, [P, c.D], F32)

    logits = nc.dram_tensor("logits", [c.Q, c.TL, c.V], BF,
                            kind="ExternalOutput")

    groups = [[g * c.SP + i for i in range(c.SP)]
              for g in range(c.n_cores // c.SP)]

    with tile.TileContext(nc) as tc:
        _emit(nc, tc, c, x_emb_in, y_emb_in, rridx, gates,
              w, opt, logits, groups, WDT)
    nc.compile()
    return nc


def _emit(nc, tc, c, x_emb_in, y_emb_in, rridx, gates,
          w, opt, logits, groups, WDT):
    from contextlib import ExitStack
    es = ExitStack()
    pool = lambda name, bufs, space="SBUF": es.enter_context(
        tc.tile_pool(name=name, bufs=bufs, space=space))

    const = pool("const", 1)
    persist = pool("persist", 1)
    wpool = pool("wpool", 5)       # lhsT-style weight chunks (qkv/ff1)
    wk = pool("wk", 7)             # rhs-style weight chunks (v/out/ff2/head)
    act = pool("act", 3)
    scratch = pool("scratch", 2)
    dram = pool("dram", 4, space="DRAM")
    ps_big = pool("ps_big", 4, space="PSUM")    # 4 banks: accum for out/ff2/v
    ps_chunk = pool("ps_chunk", 2, space="PSUM")  # 2 banks: qkv/ff1/scores/tr
    ps_att = pool("ps_att", 2, space="PSUM")    # 2 banks: AV accum

    def wdma(i, out, in_, noscalar=False):
        """Weight-stream DMA spread over the two HWDGE queues.  The gpsimd
        SWDGE queue is reserved for AllGather receives so they never queue
        behind weight traffic."""
        engs = [nc.sync, nc.scalar]
        engs[i % 2].dma_start(out=out, in_=in_)

    ISC = 1.0 / 64.0 if c.FP8 else None   # inverse weight prescale

    def evac(dsl, psl, eng, bias=None):
        """PSUM -> SBUF copy with optional 1/WSCALE and bias fold."""
        if ISC is None:
            if bias is not None:
                nc.vector.tensor_scalar(out=dsl, in0=psl, scalar1=bias,
                                        scalar2=None, op0=ALU.add)
            elif eng == "s":
                nc.scalar.copy(out=dsl, in_=psl)
            else:
                nc.vector.tensor_copy(out=dsl, in_=psl)
        else:
            if bias is not None:
                nc.vector.tensor_scalar(out=dsl, in0=psl, scalar1=ISC,
                                        scalar2=bias, op0=ALU.mult,
                                        op1=ALU.add)
            elif eng == "s":
                nc.scalar.activation(dsl, psl, AF.Copy, scale=ISC)
            else:
                nc.vector.tensor_scalar_mul(dsl, psl, ISC)

    def big_tile():
        return ps_big.tile([P, 512], F32, tag="ps_big", name="psb")

    def chunk_tile():
        t = ps_chunk.tile([P, 512], F32, tag="ps_chunk", name="psc")
        return t

    def att_tile():
        return ps_att.tile([P, 512], F32, tag="ps_att", name="psa")

    # constants
    ident_f = const.tile([P, P], F32, name="ident_f")
    make_identity(nc, ident_f[:])
    causT = const.tile([P, P], F32, name="causT")
    nc.gpsimd.memset(causT[:], 0.0)
    # transposed causal: fill -1e9 where tk > tq  (iota = tq - tk >= 0 keeps)
    nc.gpsimd.affine_select(out=causT[:], in_=causT[:],
                            compare_op=ALU.is_ge, fill=-1e9, base=0,
                            pattern=[[1, P]], channel_multiplier=-1)
    eps_t = const.tile([P, 1], F32, name="eps_t")
    nc.vector.memset(eps_t[:], 1e-5)
    gates_sb = const.tile([P, 4], F32, name="gates_sb")
    nc.sync.dma_start(out=gates_sb[:], in_=gates[:])
    rridx_sb = const.tile([P, 1], I32, name="rridx_sb")
    nc.sync.dma_start(out=rridx_sb[:], in_=rridx[:])

    opt_sb = {}
    for nm in opt:
        shp = list(opt[nm].shape)
        t = const.tile(shp, F32, name=f"sb_{nm}")
        nc.sync.dma_start(out=t[:], in_=opt[nm][:])
        opt_sb[nm] = t

    # persistent activations
    x_nat = [persist.tile([P, c.D], F32, name=f"x_{t}", tag=f"x_{t}")
             for t in range(c.NTL)]
    y_emb = [persist.tile([P, c.D], F32, name=f"y_{t}", tag=f"y_{t}")
             for t in range(c.NTL)]
    ADT = F8 if c.FP8 else BF        # fat-matmul activation dtype
    x_T = persist.tile([P, c.ND * c.TL], ADT, name="x_T", tag="x_T")
    mem_T = persist.tile([P, c.ND * c.TL], ADT, name="mem_T", tag="mem_T")
    q_T = persist.tile([P, c.ND * c.TL], BF, name="q_T", tag="q_T")
    # combined (K^T, V_aug) tiles: [own half 6144 | peer half 6144]; each
    # half is [k slot, k slot, v slot, v slot] so the AG send/recv are single
    # contiguous row blocks.
    kv = persist.tile([P, 2 * c.AGW], BF, name="kv", tag="kv")
    ckv = [persist.tile([P, 2 * c.AGW], BF, name=f"ckv{i}", tag=f"ckv{i}")
           for i in range(2)]
    y_T0 = persist.tile([P, c.ND * c.TL], ADT, name="y_T0", tag="y_T0")
    attn_T = persist.tile([P, c.ND * c.TL], ADT, name="attn_T", tag="attn_T")
    h_T = persist.tile([P, c.NFF * c.TL], ADT, name="h_T", tag="h_T")

    # ones columns of the v_aug blocks (set once; projections/recv write the
    # value cols, and the AG transports the peer's ones columns verbatim)
    for kvt in [kv] + ckv:
        for s in range(c.NT):
            blk = kvt[:, c.vcol(s):c.vcol(s) + c.WA].rearrange(
                "p (h f) -> p h f", f=P)
            nc.vector.memset(blk[:, :, 64:128], 1.0)

    # ---------------- helpers ----------------
    def transpose_tile(dst, t, srct=None):
        """dst[:, d*TL + t*P : +P] = src[t][:, d*P:+P].T (fp32 cast)."""
        srcl = x_nat if srct is None else srct
        for d in range(c.ND):
            pt = ps_chunk.tile([P, P], F32, tag="ps_chunk", name="pt")
            nc.tensor.transpose(pt[:], srcl[t][:, d * P:(d + 1) * P],
                                ident_f[:])
            dsl = dst[:, d * c.TL + t * P: d * c.TL + t * P + P]
            if (t + d) % 2 == 0:
                nc.scalar.copy(out=dsl, in_=pt[:])
            else:
                nc.vector.tensor_copy(out=dsl, in_=pt[:])

    def transpose_x_into(dst):
        for t in range(c.NTL):
            transpose_tile(dst, t)

    def embed(src_dram, dst):
        for t in range(c.NTL):
            nc.sync.dma_start(out=dst[t][:],
                              in_=src_dram[t * P:(t + 1) * P, :])

    def ln_tile(t, sub, gname, bname):
        """x_nat[t] = LN(x_nat[t] + sub) * g + b (post-norm)."""
        xt = x_nat[t]
        s1 = scratch.tile([P, 1], F32, tag="lnstat", name="s1", bufs=8)
        s2 = scratch.tile([P, 1], F32, tag="lnstat", name="s2", bufs=8)
        sq = scratch.tile([P, c.D], BF, tag="lnsq", name="sq")
        nc.vector.tensor_tensor(out=xt[:], in0=xt[:], in1=sub[:], op=ALU.add)
        nc.vector.reduce_sum(out=s1[:], in_=xt[:], axis=AX.X)
        nc.scalar.activation(sq[:], xt[:], AF.Square, accum_out=s2[:])
        mean = scratch.tile([P, 1], F32, tag="lnstat", name="mean", bufs=8)
        var = scratch.tile([P, 1], F32, tag="lnstat", name="var", bufs=8)
        m2 = scratch.tile([P, 1], F32, tag="lnstat", name="m2", bufs=8)
        nc.vector.tensor_scalar_mul(mean[:], s1[:], 1.0 / c.D)
        nc.vector.tensor_scalar_mul(var[:], s2[:], 1.0 / c.D)
        nc.vector.tensor_tensor(out=m2[:], in0=mean[:], in1=mean[:],
                                op=ALU.mult)
        nc.vector.tensor_tensor(out=var[:], in0=var[:], in1=m2[:],
                                op=ALU.subtract)
        rstd = scratch.tile([P, 1], F32, tag="lnstat", name="rstd", bufs=8)
        nc.scalar.activation(rstd[:], var[:], AF.Sqrt, bias=eps_t[:])
        nc.vector.reciprocal(rstd[:], rstd[:])
        nc.vector.tensor_scalar(out=xt[:], in0=xt[:], scalar1=mean[:],
                                scalar2=rstd[:], op0=ALU.subtract,
                                op1=ALU.mult)
        if gname in opt_sb:
            nc.vector.tensor_tensor(out=xt[:], in0=xt[:],
                                    in1=opt_sb[gname][:], op=ALU.mult)
        if bname in opt_sb:
            nc.vector.tensor_tensor(out=xt[:], in0=xt[:],
                                    in1=opt_sb[bname][:], op=ALU.add)

    def proj_qk(wname, bname, src_T, dst, kind):
        """kind='q': chunks 0-7 -> dst[:, m*TL cols]; kind='k': chunks 8-15
        -> the own key slots of the combined kv tile `dst`."""
        cbase = 0 if kind == "q" else c.ND
        for m in range(c.ND):
            wt = wpool.tile([P, c.D], WDT, tag="wqkv", name="wt")
            wdma(m, wt[:], w[wname][cbase + m], noscalar=True)
            ps = chunk_tile()[:, 0:c.TL]
            if c.FP8:
                wt3 = wt[:].rearrange("p (k m) -> p k m", m=P)
                x3 = src_T[:].rearrange("p (k t) -> p k t", t=c.TL)
                for j in range(c.ND // 2):
                    nc.tensor.matmul(ps[:], wt3[:, 2 * j:2 * j + 2, :],
                                     x3[:, 2 * j:2 * j + 2, :],
                                     start=(j == 0), stop=(j == c.ND // 2 - 1),
                                     perf_mode=DR)
            else:
                for k in range(c.ND):
                    nc.tensor.matmul(ps[:], wt[:, k * P:(k + 1) * P],
                                     src_T[:, k * c.TL:(k + 1) * c.TL],
                                     start=(k == 0), stop=(k == c.ND - 1))
            if bname in opt_sb:
                bcol = m if kind == "q" else c.ND + m
                bias = opt_sb[bname][:, bcol:bcol + 1]
            else:
                bias = None
            if kind == "q":
                evac(dst[:, m * c.TL:(m + 1) * c.TL], ps[:],
                     "v" if m % 2 == 0 else "s", bias)
            else:
                for tl in range(c.NTL):
                    dsl = dst[:, c.kcol(tl, m):c.kcol(tl, m) + P]
                    psl = ps[:, tl * P:(tl + 1) * P]
                    evac(dsl, psl, "v", bias)

    def proj_v(wname, src_T, dst_aug):
        """chunks 16-23 (rhs layout): natural v for own tiles -> dst_aug
        slots 0..NTL-1 data columns."""
        NB = c.D // 512
        pss = [[big_tile() for n in range(NB)] for t in range(c.NTL)]
        if c.FP8:
            src3 = src_T[:].rearrange("p (k t) -> p k t", t=c.TL)
            for j in range(c.ND // 2):
                wt = wk.tile([P, 2 * c.D], F8, tag="wv", name="wt")
                wt3 = wt[:].rearrange("p (k d) -> p k d", d=c.D)
                wdma(j, wt3,
                     w[wname][2 * c.ND + 2 * j:2 * c.ND + 2 * j + 2]
                     .rearrange("k p d -> p k d"))
                for t in range(c.NTL):
                    for n in range(NB):
                        nc.tensor.matmul(
                            pss[t][n][:],
                            src3[:, 2 * j:2 * j + 2, t * P:t * P + P],
                            wt3[:, :, n * 512:(n + 1) * 512],
                            start=(j == 0), stop=(j == c.ND // 2 - 1),
                            perf_mode=DR)
        else:
            for k in range(c.ND):
                wt = wk.tile([P, c.D], BF, tag="wv", name="wt")
                wdma(k, wt[:], w[wname][2 * c.ND + k], noscalar=True)
                for t in range(c.NTL):
                    for n in range(NB):
                        nc.tensor.matmul(
                            pss[t][n][:],
                            src_T[:, k * c.TL + t * P: k * c.TL + t * P + P],
                            wt[:, n * 512:(n + 1) * 512],
                            start=(k == 0), stop=(k == c.ND - 1))
        for t in range(c.NTL):
            blk = dst_aug[:, c.vcol(t):c.vcol(t) + c.WA].rearrange(
                "p (h f) -> p h f", f=P)
            for n in range(NB):
                # 512 cols = 8 heads' worth of 64-wide value blocks
                psv = pss[t][n][:].rearrange("p (h f) -> p h f", f=64)
                dstb = blk[:, n * 8:(n + 1) * 8, 0:64]
                if c.FP8:
                    nc.scalar.activation(dstb, psv, AF.Copy, scale=1.0 / 64.0)
                else:
                    nc.scalar.copy(out=dstb, in_=psv)

    KR = c.NTL * c.KSLOT            # k region width per half (2048)
    VR = c.AGW - KR

    def ag_start(kv_dst, lo, hi, tag):
        """Send + AllGather-trigger for kv_dst[:, lo:hi].  The receive is a
        SEPARATE call so triggers never queue behind earlier receives on the
        in-order gpsimd stream."""
        agin = dram.tile([P, hi - lo], BF, tag=f"agin{tag}", name="agin")
        agout = dram.tile([c.SP * P, hi - lo], BF, tag=f"agout{tag}",
                          name="agout")
        with tc.high_priority():
            nc.sync.dma_start(out=agin[:], in_=kv_dst[:, lo:hi])
        nc.gpsimd.collective_compute(
            "AllGather", ALU.bypass, replica_groups=groups,
            ins=[agin[:].opt()], outs=[agout[:].opt()])
        return (agout, kv_dst, c.AGW + lo)

    def ag_recv(h):
        agout, kv_dst, dst0 = h
        n = agout.shape[-1]
        nc.gpsimd.indirect_dma_start(
            out=kv_dst[:, dst0:dst0 + n],
            out_offset=None,
            in_=agout[:],
            in_offset=IndirectOffsetOnAxis(ap=rridx_sb[:, :1], axis=0))

    def ag_k_start(kv_dst):
        return ag_start(kv_dst, 0, KR, "k")

    def ag_v_start(kv_dst):
        return ag_start(kv_dst, KR, c.AGW, "v")

    def attention(kvt, mode):
        """q_T x kvt -> attn_T.  mode: 'full' (all 4 slots, no mask) or
        'causal' (slot structure for decoder self-attention)."""
        for h in range(c.H):
            m = h // 2
            po = 64 * (h % 2)
            # score slot pairs share one PSUM bank + one exp where possible:
            # pair block cols [s%2 * TL : +TL]; in causal mode slot 1 only
            # has q cols [P:TL] (placed at [TL+P:2TL]) and slots 0/1 get the
            # diagonal causT mask; slots 2/3 share one gate bias.
            at = []
            for pi, (s0, s1) in enumerate([(0, 1), (2, 3)]):
                pss = big_tile()
                a = act.tile([P, 2 * c.TL], BF, tag=f"ATp{pi}", name="at",
                             bufs=3)
                qr = []
                for si, s in enumerate((s0, s1)):
                    q0 = P if (mode == "causal" and s == 1) else 0
                    base = si * c.TL
                    nc.tensor.matmul(
                        pss[:, base + q0: base + c.TL],
                        kvt[po:po + 64, c.kcol(s, m):c.kcol(s, m) + P],
                        q_T[po:po + 64, m * c.TL + q0: m * c.TL + c.TL],
                        start=True, stop=True)
                    if mode == "causal" and s < 2:
                        # diagonal block: q columns s*P of this slot
                        d0 = base + s * P
                        nc.vector.tensor_tensor(
                            out=pss[:, d0:d0 + P], in0=pss[:, d0:d0 + P],
                            in1=causT[:], op=ALU.add)
                    qr.append(q0)
                if mode == "causal" and pi == 1:
                    nc.scalar.activation(a[:], pss[:], AF.Exp, scale=c.SCALE,
                                         bias=gates_sb[:, s0:s0 + 1])
                elif mode == "causal":
                    # two exps: slot0 full, slot1 partial (skip the dead gap)
                    nc.scalar.activation(a[:, 0:c.TL], pss[:, 0:c.TL],
                                         AF.Exp, scale=c.SCALE)
                    nc.scalar.activation(a[:, c.TL + P:2 * c.TL],
                                         pss[:, c.TL + P:2 * c.TL],
                                         AF.Exp, scale=c.SCALE)
                else:
                    nc.scalar.activation(a[:], pss[:], AF.Exp, scale=c.SCALE)
                at.append((s0, a, 0, qr[0]))
                at.append((s1, a, c.TL, qr[1]))
            ps_o = att_tile()[:, 0:c.TL]
            for i, (s, a, base, q0) in enumerate(at):
                nc.tensor.matmul(
                    ps_o[:, q0:c.TL],
                    kvt[:, c.vcol(s) + h * P: c.vcol(s) + (h + 1) * P],
                    a[:, base + q0: base + c.TL],
                    start=(i == 0), stop=(i == len(at) - 1))
            rden = scratch.tile([64, c.TL], F32, tag="rden", name="rden",
                                bufs=2)
            nc.vector.reciprocal(rden[:], ps_o[64:128, :])
            nc.vector.tensor_tensor(
                out=attn_T[po:po + 64, m * c.TL:(m + 1) * c.TL],
                in0=ps_o[0:64, :], in1=rden[:], op=ALU.mult)

    def mm_to_natural(src_T, nk, wname, bname, noscalar=False):
        """[TL, D] = src_T.T @ W (k-chunk streaming, PSUM accumulate),
        returned as per-t bf16 [P, D] tiles."""
        NB = c.D // 512
        pss = [[big_tile() for n in range(NB)] for t in range(c.NTL)]
        if c.FP8:
            src3 = src_T[:].rearrange("p (k t) -> p k t", t=c.TL)
            for j in range(nk // 2):
                wt = wk.tile([P, 2 * c.D], F8, tag="wnat", name="wt")
                wt3 = wt[:].rearrange("p (k d) -> p k d", d=c.D)
                wdma(j, wt3,
                     w[wname][2 * j:2 * j + 2].rearrange("k p d -> p k d"))
                for t in range(c.NTL):
                    for n in range(NB):
                        nc.tensor.matmul(
                            pss[t][n][:],
                            src3[:, 2 * j:2 * j + 2, t * P:t * P + P],
                            wt3[:, :, n * 512:(n + 1) * 512],
                            start=(j == 0), stop=(j == nk // 2 - 1),
                            perf_mode=DR)
        else:
            for k in range(nk):
                wt = wk.tile([P, c.D], BF, tag="wnat", name="wt")
                wdma(k, wt[:], w[wname][k], noscalar=noscalar)
                for t in range(c.NTL):
                    for n in range(NB):
                        nc.tensor.matmul(
                            pss[t][n][:],
                            src_T[:, k * c.TL + t * P: k * c.TL + t * P + P],
                            wt[:, n * 512:(n + 1) * 512],
                            start=(k == 0), stop=(k == nk - 1))
        parts = []
        for t in range(c.NTL):
            sb = scratch.tile([P, c.D], BF, tag="oproj", name="sb", bufs=3)
            for n in range(NB):
                evac(sb[:, n * 512:(n + 1) * 512], pss[t][n][:],
                     "s" if (t + n) % 2 == 0 else "v")
            if bname in opt_sb:
                nc.vector.tensor_tensor(out=sb[:], in0=sb[:],
                                        in1=opt_sb[bname][:], op=ALU.add)
            parts.append(sb)
        return parts

    def ffn(w1name, b1name, w2name, b2name):
        x3 = x_T[:].rearrange("p (k t) -> p k t", t=c.TL)
        for mchunk in range(c.NFF):
            wt = wpool.tile([P, c.D], WDT, tag="wff1", name="wt")
            wdma(mchunk, wt[:], w[w1name][mchunk])
            ps = chunk_tile()[:, 0:c.TL]
            if c.FP8:
                wt3 = wt[:].rearrange("p (k m) -> p k m", m=P)
                for j in range(c.ND // 2):
                    nc.tensor.matmul(ps[:], wt3[:, 2 * j:2 * j + 2, :],
                                     x3[:, 2 * j:2 * j + 2, :],
                                     start=(j == 0), stop=(j == c.ND // 2 - 1),
                                     perf_mode=DR)
            else:
                for k in range(c.ND):
                    nc.tensor.matmul(ps[:], wt[:, k * P:(k + 1) * P],
                                     x_T[:, k * c.TL:(k + 1) * c.TL],
                                     start=(k == 0), stop=(k == c.ND - 1))
            dsl = h_T[:, mchunk * c.TL:(mchunk + 1) * c.TL]
            kw = {"scale": 1.0 / 64.0} if c.FP8 else {}
            if b1name in opt_sb:
                nc.scalar.activation(dsl, ps[:], AF.Relu,
                                     bias=opt_sb[b1name][:, mchunk:mchunk + 1],
                                     **kw)
            else:
                nc.scalar.activation(dsl, ps[:], AF.Relu, **kw)
        return mm_to_natural(h_T, c.NFF, w2name, b2name)

    def ag_warm(dep_tile):
        """Tiny AllGather that wakes the cc stream ahead of a real AG;
        dep_tile pins its position in the schedule."""
        n = dep_tile.shape[-1]
        win = dram.tile([P, n], dep_tile.dtype, tag="warmin", name="win")
        wout = dram.tile([c.SP * P, n], dep_tile.dtype, tag="warmout",
                         name="wout")
        nc.sync.dma_start(out=win[:], in_=dep_tile)
        nc.gpsimd.collective_compute(
            "AllGather", ALU.bypass, replica_groups=groups,
            ins=[win[:].opt()], outs=[wout[:].opt()])

    def cross_kv_proj(l):
        buf = l % 2
        proj_qk(f"d_cqkv_{l}", f"d_cqkv_b_{l}", mem_T, ckv[buf], "k")
        proj_v(f"d_cqkv_{l}", mem_T, ckv[buf])

    def cross_kv_prefetch(l):
        """Project + AllGather cross-attention K/V for decoder layer l."""
        cross_kv_proj(l)
        hk = ag_k_start(ckv[l % 2])
        hv = ag_v_start(ckv[l % 2])
        ag_recv(hk)
        ag_recv(hv)

    # ---------------- encoder ----------------
    ag_warm(gates_sb[:, 0:4])
    embed(x_emb_in, x_nat)
    embed(y_emb_in, y_emb)
    transpose_x_into(x_T)
    for l in range(c.L):
        proj_qk(f"e_qkv_{l}", f"e_qkv_b_{l}", x_T, kv, "k")
        if l > 0:
            ag_warm(kv[:, 0:8])
        hk = ag_k_start(kv)
        proj_v(f"e_qkv_{l}", x_T, kv)
        hv = ag_v_start(kv)
        hek = hev = None
        if l == 0:
            # decoder layer-0 self K/V depend only on the target embedding:
            # project + gather them now, into the idle ckv[1] buffer, so the
            # PE work and cc traffic fill the encoder warm-up stalls
            for t in range(c.NTL):
                transpose_tile(y_T0, t, srct=y_emb)
            proj_qk("d_sqkv_0", "d_sqkv_b_0", y_T0, ckv[1], "k")
            hek = ag_k_start(ckv[1])
            proj_v("d_sqkv_0", y_T0, ckv[1])
            hev = ag_v_start(ckv[1])
        ag_recv(hk)
        ag_recv(hv)
        if hek is not None:
            ag_recv(hek)
            ag_recv(hev)
        proj_qk(f"e_qkv_{l}", f"e_qkv_b_{l}", x_T, q_T, "q")
        attention(kv, "full")
        parts = mm_to_natural(attn_T, c.ND, f"e_out_{l}", f"e_out_b_{l}",
                              noscalar=True)
        for t in range(c.NTL):
            ln_tile(t, parts[t], f"e_ln1_w_{l}", f"e_ln1_b_{l}")
            transpose_tile(x_T, t)
        parts = ffn(f"e_ff1_{l}", f"e_ff1_b_{l}", f"e_ff2_{l}", f"e_ff2_b_{l}")
        last = l == c.L - 1
        for t in range(c.NTL):
            ln_tile(t, parts[t], f"e_ln2_w_{l}", f"e_ln2_b_{l}")
            transpose_tile(mem_T if last else x_T, t)

    # cross K/V for decoder layer 0 (hides under decoder embed + self attn)
    cross_kv_prefetch(0)

    # ---------------- decoder ----------------
    for t in range(c.NTL):
        nc.vector.tensor_copy(out=x_nat[t][:], in_=y_emb[t][:])
        transpose_tile(x_T, t)
    for l in range(c.L):
        if l > 0:
            proj_qk(f"d_sqkv_{l}", f"d_sqkv_b_{l}", x_T, kv, "k")
            ag_warm(kv[:, 0:8])
            hk = ag_k_start(kv)
            proj_v(f"d_sqkv_{l}", x_T, kv)
            hv = ag_v_start(kv)
            hck = hcv = None
            if l + 1 < c.L:
                cross_kv_proj(l + 1)
                hck = ag_k_start(ckv[(l + 1) % 2])
                hcv = ag_v_start(ckv[(l + 1) % 2])
            ag_recv(hk)
            ag_recv(hv)
            if hck is not None:
                ag_recv(hck)
                ag_recv(hcv)
            proj_qk(f"d_sqkv_{l}", f"d_sqkv_b_{l}", x_T, q_T, "q")
            attention(kv, "causal")
        else:
            # layer-0 self K/V were projected + gathered during the encoder;
            # consume them from ckv[1] BEFORE cross_kv_proj(1) recycles it
            ag_warm(x_T[:, 0:8])
            proj_qk("d_sqkv_0", "d_sqkv_b_0", x_T, q_T, "q")
            attention(ckv[1], "causal")
            cross_kv_proj(1)
            hck = ag_k_start(ckv[1])
            hcv = ag_v_start(ckv[1])
            ag_recv(hck)
            ag_recv(hcv)
        parts = mm_to_natural(attn_T, c.ND, f"d_sout_{l}", f"d_sout_b_{l}",
                              noscalar=True)
        for t in range(c.NTL):
            ln_tile(t, parts[t], f"d_ln1_w_{l}", f"d_ln1_b_{l}")
            transpose_tile(x_T, t)
        proj_qk(f"d_cqkv_{l}", f"d_cqkv_b_{l}", x_T, q_T, "q")
        attention(ckv[l % 2], "full")
        parts = mm_to_natural(attn_T, c.ND, f"d_cout_{l}", f"d_cout_b_{l}",
                              noscalar=True)
        for t in range(c.NTL):
            ln_tile(t, parts[t], f"d_ln2_w_{l}", f"d_ln2_b_{l}")
            transpose_tile(x_T, t)
        parts = ffn(f"d_ff1_{l}", f"d_ff1_b_{l}", f"d_ff2_{l}", f"d_ff2_b_{l}")
        for t in range(c.NTL):
            ln_tile(t, parts[t], f"d_ln3_w_{l}", f"d_ln3_b_{l}")
            transpose_tile(x_T, t)

    # ---------------- output head ----------------
    NBV = c.V // 512
    for j in range(c.Q):
        hw = wk.tile([P, c.V], WDT, tag="whead", name="hw", bufs=2)
        wdma(j, hw[:], w["head_t"][j])
        for t in range(c.NTL):
            sb = scratch.tile([P, c.V], BF, tag="lgt", name="sb", bufs=3)
            for n in range(NBV):
                ps = big_tile()
                nc.tensor.matmul(
                    ps[:], x_T[:, j * c.TL + t * P: j * c.TL + t * P + P],
                    hw[:, n * 512:(n + 1) * 512], start=True, stop=True)
                evac(sb[:, n * 512:(n + 1) * 512], ps[:], "s")
            if "head_b" in opt_sb:
                nc.vector.tensor_tensor(
                    out=sb[:], in0=sb[:],
                    in1=opt_sb["head_b"][:, j * c.V:(j + 1) * c.V],
                    op=ALU.add)
            nc.sync.dma_start(out=logits[j, t * P:(t + 1) * P, :], in_=sb[:])

    es.close()


# --------------------------------------------------------------------------
# host side
# --------------------------------------------------------------------------

_PROG_CACHE = {}


def parse_cfg(inputs, n_cores=8, fp8=None):
    B, Q, T = inputs["input_codes"].shape
    _, V, E = np.asarray(inputs["tok_emb"]).shape
    L, _, D = np.asarray(inputs["e_qkv_w"]).shape
    FF = np.asarray(inputs["e_ff1_w"]).shape[1]
    H = D // 64
    flags = set()
    for l in range(L):
        for knm in ["e_qkv_b", "d_sqkv_b", "d_cqkv_b", "e_ff1_b", "d_ff1_b",
                    "e_out_b", "e_ff2_b", "d_sout_b", "d_cout_b", "d_ff2_b"]:
            if np.any(np.asarray(inputs[knm])[l]):
                flags.add(f"{knm}_{l}")
        for ln in ["e_ln1", "e_ln2", "d_ln1", "d_ln2", "d_ln3"]:
            if not np.all(np.asarray(inputs[ln + "_w"])[l] == 1.0):
                flags.add(f"{ln}_w_{l}")
            if np.any(np.asarray(inputs[ln + "_b"])[l]):
                flags.add(f"{ln}_b_{l}")
    if np.any(np.asarray(inputs["head_b"])):
        flags.add("head_b")
    if fp8 is None:
        fp8 = os.environ.get("BASS_S2S_FP8", "0") == "1"
    # v-bias unsupported in-kernel; fall back assertion
    for l in range(L):
        for nm in ["e_qkv_b", "d_sqkv_b", "d_cqkv_b"]:
            vb = np.asarray(inputs[nm])[l][2 * D:3 * D]
            assert not np.any(vb), "nonzero v bias not supported"
    return Cfg(B, Q, T, D, H, V, L, FF, n_cores, flags, fp8=fp8)


def _lhsT_chunks(wm, D):
    """[M, D] row-major weight -> [M//128, 128(p=in%128), ...] lhsT chunk
    layout: chunk c element [p, nd*128 + m] = wm[c*128 + m, nd*128 + p]."""
    M = wm.shape[0]
    nd = D // P
    out = np.empty((M // P, P, D), np.float32)
    for cc in range(M // P):
        wc = wm[cc * P:(cc + 1) * P, :]          # [128 m, D in]
        out[cc] = wc.T.reshape(nd, P, P).transpose(1, 0, 2).reshape(P, D)
    return out


def _rhs_chunks(wm, D_out):
    """[D_out, K] row-major weight -> [K//128, 128(p=k%128), D_out] rhs
    chunk layout: chunk k element [p, n] = wm[n, k*128 + p]."""
    K = wm.shape[1]
    return np.ascontiguousarray(
        wm.T.reshape(K // P, P, D_out))


def build_inmaps(inputs, c: Cfg):
    g = lambda nm: np.asarray(inputs[nm], np.float32)
    if c.FP8:
        def bf(a):
            a = np.ascontiguousarray(a, dtype=np.float32) * c.WSCALE
            return np.clip(a, -240.0, 240.0).astype(FP8NP)
    else:
        bf = lambda a: np.ascontiguousarray(a, dtype=np.float32).astype(BF16)

    tok = np.asarray(inputs["tok_emb"], np.float32)
    posf = np.ascontiguousarray(g("pos_emb")[0, :c.T, :])
    head_w = g("head_w")

    common = {}
    head_t = np.stack([head_w[q].T for q in range(c.Q)])    # [Q, E, V]
    common["head_t"] = bf(head_t)
    if "head_b" in c.flags:
        hb = g("head_b").reshape(-1)
        common["head_b"] = np.broadcast_to(hb, (P, c.Q * c.V)).copy()

    for pre, wq, wo in [("e_qkv", "e_qkv_w", None), ("e_out", None, "e_out_w"),
                        ("d_sqkv", "d_sqkv_w", None),
                        ("d_sout", None, "d_sout_w"),
                        ("d_cqkv", "d_cqkv_w", None),
                        ("d_cout", None, "d_cout_w")]:
        for l in range(c.L):
            if wq is not None:
                qkv = g(wq)[l]                    # [3D, D]
                qk = _lhsT_chunks(qkv[0:2 * c.D], c.D)      # q,k chunks
                vv = _rhs_chunks(qkv[2 * c.D:3 * c.D], c.D)
                common[f"{pre}_{l}"] = bf(np.concatenate([qk, vv], axis=0))
            else:
                wo_l = g(wo)[l]                   # [D, D] rows = out dim
                common[f"{pre}_{l}"] = bf(_rhs_chunks(wo_l, c.D))
    for l in range(c.L):
        common[f"e_ff1_{l}"] = bf(_lhsT_chunks(g("e_ff1_w")[l], c.D))
        common[f"d_ff1_{l}"] = bf(_lhsT_chunks(g("d_ff1_w")[l], c.D))
        common[f"e_ff2_{l}"] = bf(_rhs_chunks(g("e_ff2_w")[l], c.D))
        common[f"d_ff2_{l}"] = bf(_rhs_chunks(g("d_ff2_w")[l], c.D))

    # optional biases
    for l in range(c.L):
        for knm in ["e_qkv_b", "d_sqkv_b", "d_cqkv_b"]:
            if f"{knm}_{l}" in c.flags:
                b = g(knm)[l][0:2 * c.D]           # q,k bias only
                common[f"{knm}_{l}"] = np.ascontiguousarray(
                    b.reshape(2 * c.ND, P).T)
        for knm in ["e_ff1_b", "d_ff1_b"]:
            if f"{knm}_{l}" in c.flags:
                common[f"{knm}_{l}"] = np.ascontiguousarray(
                    g(knm)[l].reshape(c.NFF, P).T)
        for knm in ["e_out_b", "e_ff2_b", "d_sout_b", "d_cout_b", "d_ff2_b"]:
            if f"{knm}_{l}" in c.flags:
                common[f"{knm}_{l}"] = np.broadcast_to(
                    g(knm)[l], (P, c.D)).copy()
        for ln in ["e_ln1", "e_ln2", "d_ln1", "d_ln2", "d_ln3"]:
            for sfx in ["w", "b"]:
                if f"{ln}_{sfx}_{l}" in c.flags:
                    common[f"{ln}_{sfx}_{l}"] = np.broadcast_to(
                        g(f"{ln}_{sfx}")[l], (P, c.D)).copy()

    codes_in = np.asarray(inputs["input_codes"], np.int32)
    codes_tgt = np.asarray(inputs["target_codes"], np.int32)

    def embed_host(codes_bqt):
        # [Q, T] codes -> [T, D] embedding (concat per-quantizer) + pos
        e = np.concatenate([tok[q][codes_bqt[q]] for q in range(c.Q)],
                           axis=-1)
        return e + posf

    emb_in = [embed_host(codes_in[b]) for b in range(c.B)]
    emb_tgt = [embed_host(codes_tgt[b]) for b in range(c.B)]
    in_maps = []
    for core in range(c.n_cores):
        b, h = core // c.SP, core % c.SP
        m = dict(common)
        sl = slice(h * c.TL, (h + 1) * c.TL)
        m["x_emb"] = np.ascontiguousarray(emb_in[b % c.B][sl])
        m["y_emb"] = np.ascontiguousarray(emb_tgt[b % c.B][sl])
        m["rridx"] = ((1 - h) * P + np.arange(P, dtype=np.int32)
                      ).reshape(P, 1)
        gate = np.zeros((P, 4), np.float32)
        if h == 0:
            gate[:, 2] = -1e9
            gate[:, 3] = -1e9
        m["gates"] = gate
        in_maps.append(m)
    return in_maps


def postprocess(results, c: Cfg):
    out = np.empty((c.B, c.T, c.Q, c.V), np.float32)
    for b in range(c.B):
        for h in range(c.SP):
            r = np.asarray(results[b * c.SP + h]["logits"], np.float32)
            out[b, h * c.TL:(h + 1) * c.TL] = r.transpose(1, 0, 2)
    return out


def run(inputs, trace=False):
    from concourse.bass_utils import run_bass_kernel_spmd
    c = parse_cfg(inputs)
    key = c.key()
    if key not in _PROG_CACHE:
        _PROG_CACHE[key] = build_program(c)
    nc = _PROG_CACHE[key]
    in_maps = build_inmaps(inputs, c)
    res = run_bass_kernel_spmd(nc, in_maps, list(range(c.n_cores)),
                               trace=trace)
    return postprocess(res.results, c), res


def kernel(**inputs):
    out, _ = run(inputs, trace=False)
    return out
